# revision 25
# baseline (speedup 1.0000x reference)
"""Self-contained Trainium2 Bass kernel for the CrossAttention problem.

Shapes (hardcoded): B=4, L=2048, D=512, H=8, Dh=64, PF=2048.
Sharding: 8 cores = 2 inputs (question/query) x 4 batches. No collectives --
each core computes K/V projections for BOTH inputs of its batch, builds the
shared linear-attention state S = K1^T V1 + K2^T V2 (identical for both
sides), then uses the fold
    z = x + O = x @ (Wq^T S_bd Wo^T + I) + (bq^T S_bd Wo^T + bo)
so Q-projection, attention application, out-projection AND the residual
collapse into a single D x D GEMM. LayerNorm1's gamma/beta are folded into
W1/b1 on the host (z_t carries the un-scaled normalized activations); the
FFN residual y1 = g*n1 + b re-enters the second FFN GEMM as an extra
diag(g) matmul so the whole z2 = y1 + ff lands in one PSUM accumulation.
Everything runs in bf16 (fp32 PSUM); LN statistics are ones-matmuls.
"""
import sys

for _p in ("/opt/trn_rl_repo", "/root/.axon_site/_ro/trn_rl_repo"):
    if _p not in sys.path:
        sys.path.insert(0, _p)

import numpy as np
import ml_dtypes

import concourse.bass as bass
import concourse.bacc as bacc
import concourse.tile as tile
from concourse import bass_utils, mybir

B = 4
L = 2048
D = 512
H = 8
DH = 64
PF = 2048
P = 128
NS = 512            # l-slice width (psum bank / fp32 moving-operand max)
NSL = L // NS       # 4 slices
DC = D // P         # 4 feature chunks
PFC = PF // P       # 16 pf chunks
EPS = 1e-5
INV_D = 1.0 / D

f32 = mybir.dt.float32
bf16 = mybir.dt.bfloat16
AF = mybir.ActivationFunctionType
OP = mybir.AluOpType

_CACHE = {}


def _build_nc(dbg=False, repeat=1):
    nc = bacc.Bacc("TRN2", target_bir_lowering=False, debug=False,
                   num_devices=8)

    def din(name, shape, dt=bf16):
        return nc.dram_tensor(name, shape, dt, kind="ExternalInput").ap()

    x_own = din("x_own", [D, L])
    x_oth = din("x_oth", [D, L])
    wkT = din("wkT", [D, D])     # Wk.T
    wvT = din("wvT", [D, D])     # Wv.T
    woT = din("woT", [D, D])     # Wo.T
    wqN = din("wqN", [D, D])     # Wq natural
    w1T = din("w1T", [D, PF])    # (W1 * ln_g).T
    w2T = din("w2T", [PF, D])    # W2.T
    idm = din("idm", [P, P])     # 128x128 identity (bf16)
    gdg = din("gdg", [D, P])     # chunked diag(ln_g) (bf16)
    bqc = din("bqc", [D])        # bq in bf16 (fold bias matmul operand)
    bk_d = din("bk", [D], f32)
    bv_d = din("bv", [D], f32)
    bo_d = din("bo", [D], f32)
    b1_d = din("b1", [PF], f32)  # W1 @ ln_b + b1
    b2_d = din("b2", [D], f32)   # b2 + ln_b
    lng_d = din("lng", [D], f32)
    lnb_d = din("lnb", [D], f32)
    yT = nc.dram_tensor("yT", [D, L], f32, kind="ExternalOutput").ap()
    if dbg:
        dWF = nc.dram_tensor("dWF", [D, D], bf16, kind="ExternalOutput").ap()
        dZ = nc.dram_tensor("dZ", [D, L], bf16, kind="ExternalOutput").ap()

    def bcast_row(v, n):
        # [n] dram vector -> [P, n] all partitions identical
        return bass.AP(tensor=v.tensor, offset=v.offset, ap=[[0, P], [1, n]])

    def chunked_col(v, nch):
        # [nch*P] dram vector -> [P, nch] (chunk c in column c)
        return bass.AP(tensor=v.tensor, offset=v.offset,
                       ap=[[1, P], [P, nch]])

    def chunk3(m, ncols, nch, c0=0, w=None):
        # [nch*P, ncols] dram matrix -> [P, nch, w] AP (row-chunk c in dim 1)
        w = ncols if w is None else w
        return bass.AP(tensor=m.tensor, offset=m.offset + c0,
                       ap=[[ncols, P], [P * ncols, nch], [1, w]])

    with tile.TileContext(nc) as tc:
      import contextlib
      rep_ctx = (tc.For_i(0, repeat, 1) if repeat > 1
                 else contextlib.nullcontext())
      with rep_ctx:
        gp = tc.alloc_tile_pool(name="gp", bufs=1)
        # ---- persistent tiles -------------------------------------------
        z_t = gp.tile([P, DC, L], bf16, name="z_t")          # 16KB (z / n1)
        w1_t = gp.tile([P, DC, PF], bf16, name="w1_t")       # 16KB
        w2_t = gp.tile([P, PFC, D], bf16, name="w2_t")       # 16KB
        s_blkT = gp.tile([P, DC, P], bf16, name="s_blkT")    # 1KB
        gd_t = gp.tile([P, DC, P], bf16, name="gd_t")        # diag(ln_g)
        ones_bf = gp.tile([P, P], bf16, name="ones_bf")
        eps_t = gp.tile([P, 1], f32, name="eps_t")
        bkb = gp.tile([P, D], f32, name="bkb")
        bvb = gp.tile([P, D], f32, name="bvb")
        bo4 = gp.tile([P, DC], f32, name="bo4")
        b24 = gp.tile([P, DC], f32, name="b24")
        g4 = gp.tile([P, DC], f32, name="g4")
        lnb4 = gp.tile([P, DC], f32, name="lnb4")
        b116 = gp.tile([P, PFC], f32, name="b116")
        bq4b = gp.tile([P, DC], bf16, name="bq4b")
        bfold4 = gp.tile([P, DC], f32, name="bfold4")

        # -------- stage A pools (declared before first DMAs need them) ----
        pab = tc.alloc_tile_pool(name="pab", bufs=1)
        wf_t = pab.tile([P, DC, D], bf16, name="wf_t")       # 4KB, lives A+B

        pa = tc.alloc_tile_pool(name="pa", bufs=1)
        pa2 = tc.alloc_tile_pool(name="pa2", bufs=2)
        ps_s = tc.alloc_tile_pool(name="ps_s", bufs=1, space="PSUM")
        ps_kv = tc.alloc_tile_pool(name="ps_kv", bufs=2, space="PSUM")

        wk_t = pa.tile([P, DC, D], bf16, name="wk_t")
        wv_t = pa.tile([P, DC, D], bf16, name="wv_t")
        wo_t = pa.tile([P, DC, D], bf16, name="wo_t")
        wq_t = pa.tile([P, DC, D], bf16, name="wq_t")
        m1_t = pa.tile([P, DC, D], bf16, name="m1_t")
        id_t = pa.tile([P, P], bf16, name="id_t")

        # critical-path loads first, on the SP HWDGE queue (single 3D DMAs)
        nc.sync.dma_start(out=wk_t, in_=chunk3(wkT, D, DC))
        # small consts on the Activation HWDGE queue (negligible traffic)
        nc.scalar.dma_start(out=bo4, in_=chunked_col(bo_d, DC))
        nc.scalar.dma_start(out=b24, in_=chunked_col(b2_d, DC))
        nc.scalar.dma_start(out=g4, in_=chunked_col(lng_d, DC))
        nc.scalar.dma_start(out=lnb4, in_=chunked_col(lnb_d, DC))
        nc.scalar.dma_start(out=b116, in_=chunked_col(b1_d, PFC))
        nc.scalar.dma_start(out=bq4b, in_=chunked_col(bqc, DC))
        nc.scalar.dma_start(out=bkb, in_=bcast_row(bk_d, D))
        nc.scalar.dma_start(out=bvb, in_=bcast_row(bv_d, D))
        nc.vector.memset(ones_bf, 1.0)
        nc.vector.memset(eps_t, EPS)

        def ln_block(ptmp, ps_st, zsl, zqsl, apply_fn, c0=0, w=NS,
                     st_ps=None):
            """LN stats for (part of) one l-slice + caller-supplied apply.

            zsl(d)/zqsl(d) -> [P, w] APs of z and z^2 chunks. apply_fn
            receives (rstd, mhat) fp32 [P, w] APs. c0/w select a column
            sub-range so a slice can be split for tail-latency."""
            if st_ps is None:
                st_ps = ps_st.tile([P, 2, NS], f32, tag="stps", name="st_ps")
            for d in range(DC):
                nc.tensor.matmul(st_ps[:, 0, c0:c0 + w], ones_bf, zsl(d),
                                 start=(d == 0), stop=(d == DC - 1))
            for d in range(DC):
                nc.tensor.matmul(st_ps[:, 1, c0:c0 + w], ones_bf, zqsl(d),
                                 start=(d == 0), stop=(d == DC - 1))
            mu2 = ptmp.tile([P, NS], f32, tag="mu2", name="mu2")[:, 0:w]
            nc.scalar.activation(out=mu2, in_=st_ps[:, 0, c0:c0 + w],
                                 func=AF.Square, scale=INV_D)
            vt = ptmp.tile([P, NS], f32, tag="vt", name="vt")[:, 0:w]
            nc.vector.scalar_tensor_tensor(out=vt,
                                           in0=st_ps[:, 1, c0:c0 + w],
                                           scalar=INV_D, in1=mu2,
                                           op0=OP.mult, op1=OP.subtract)
            nc.scalar.activation(out=vt, in_=vt, func=AF.Sqrt, bias=eps_t)
            rstd = ptmp.tile([P, NS], f32, tag="rstd", name="rstd")[:, 0:w]
            nc.vector.reciprocal(out=rstd, in_=vt)
            mhat = ptmp.tile([P, NS], f32, tag="mhat", name="mhat")[:, 0:w]
            nc.vector.scalar_tensor_tensor(out=mhat,
                                           in0=st_ps[:, 0, c0:c0 + w],
                                           scalar=INV_D, in1=rstd,
                                           op0=OP.mult, op1=OP.mult)
            apply_fn(rstd, mhat)

        # ================= Stage A: K/V + S^T + fold ======================
        s_ps = ps_s.tile([P, DC, P], f32, name="s_ps")
        sctr = 0
        n_s_groups = NSL * 4 * 2  # l-chunks * inputs

        def emit_s_group(k_sb, v_sb):
            # S^T matmuls for one (input, l-chunk) group, head-pair packed:
            # s_ps[:, c, :] = V_pair^T @ K_pair (diag 64x64 blocks are S^T_h).
            nonlocal sctr
            first, last = sctr == 0, sctr == n_s_groups - 1
            sctr += 1
            for c in range(DC):
                nc.tensor.matmul(
                    s_ps[:, c, :],
                    v_sb[:, c * P:(c + 1) * P],
                    k_sb[:, c * P:(c + 1) * P],
                    start=(first and c == 0), stop=last)

        pending = None
        xo_list = []
        for n in range(NSL):
            xo_s = pab.tile([P, DC, NS], bf16, tag="xo", bufs=4, name="xo_s")
            xo_list.append(xo_s)
            xt_s = pa2.tile([P, DC, NS], bf16, tag="xt", bufs=3, name="xt_s")
            if n == 0:
                # jj-granular loads so the very first K matmul group only
                # waits for wk + one 128-column chunk of x_own
                for jj in range(4):
                    nc.sync.dma_start(
                        out=xo_s[:, :, jj * P:(jj + 1) * P],
                        in_=chunk3(x_own, L, DC, c0=jj * P, w=P))
                    if jj == 0:
                        nc.sync.dma_start(out=wv_t, in_=chunk3(wvT, D, DC))
                    nc.sync.dma_start(
                        out=xt_s[:, :, jj * P:(jj + 1) * P],
                        in_=chunk3(x_oth, L, DC, c0=jj * P, w=P))
            else:
                nc.sync.dma_start(out=xo_s, in_=chunk3(x_own, L, DC,
                                                       c0=n * NS, w=NS))
                nc.sync.dma_start(out=xt_s, in_=chunk3(x_oth, L, DC,
                                                       c0=n * NS, w=NS))
            for jj in range(4):
                for xs in (xo_s, xt_s):
                    k_ps = ps_kv.tile([P, D], f32, tag="kps", name="k_ps")
                    v_ps = ps_kv.tile([P, D], f32, tag="vps", name="v_ps")
                    for d in range(DC):
                        nc.tensor.matmul(k_ps, xs[:, d, jj * P:(jj + 1) * P],
                                         wk_t[:, d, :],
                                         start=(d == 0), stop=(d == DC - 1))
                    for d in range(DC):
                        nc.tensor.matmul(v_ps, xs[:, d, jj * P:(jj + 1) * P],
                                         wv_t[:, d, :],
                                         start=(d == 0), stop=(d == DC - 1))
                    k_sb = pa2.tile([P, D], bf16, tag="ksb", bufs=3,
                                    name="k_sb")
                    v_sb = pa2.tile([P, D], bf16, tag="vsb", bufs=3,
                                    name="v_sb")
                    nc.vector.tensor_tensor(out=k_sb, in0=k_ps, in1=bkb,
                                            op=OP.add)
                    nc.vector.tensor_tensor(out=v_sb, in0=v_ps, in1=bvb,
                                            op=OP.add)
                    if pending is not None:
                        emit_s_group(*pending)
                    pending = (k_sb, v_sb)
        emit_s_group(*pending)

        # weights for later stages: on the SP queue AFTER all x traffic so
        # they can never steal HBM bandwidth from the stage-A critical path
        nc.sync.dma_start(out=id_t, in_=idm)
        nc.sync.dma_start(out=gd_t, in_=chunk3(gdg, P, DC))
        nc.sync.dma_start(out=wo_t, in_=chunk3(woT, D, DC))
        nc.sync.dma_start(out=wq_t, in_=chunk3(wqN, D, DC))
        nc.sync.dma_start(out=w1_t, in_=chunk3(w1T, PF, DC))
        nc.sync.dma_start(out=w2_t, in_=chunk3(w2T, D, PFC))

        # assemble block-diagonal S^T (keep only the per-head 64x64 blocks)
        nc.vector.memset(s_blkT, 0.0)
        for c in range(DC):
            nc.vector.tensor_copy(out=s_blkT[0:64, c, 0:64],
                                  in_=s_ps[0:64, c, 0:64])
            nc.vector.tensor_copy(out=s_blkT[64:128, c, 64:128],
                                  in_=s_ps[64:128, c, 64:128])

        # fold: M1 = S Wo^T ; Wfold = Wq^T M1 + I ; bfold = M1^T bq + bo
        ps_kv.release()
        ps_f = tc.alloc_tile_pool(name="ps_f", bufs=2, space="PSUM")
        for c in range(DC):
            m1_ps = ps_f.tile([P, D], f32, tag="m1", name="m1_ps")
            nc.tensor.matmul(m1_ps, s_blkT[:, c, :], wo_t[:, c, :],
                             start=True, stop=True)
            nc.scalar.activation(out=m1_t[:, c, :], in_=m1_ps,
                                 func=AF.Identity)
        for i in range(DC):
            wf_ps = ps_f.tile([P, D], f32, tag="wf", name="wf_ps")
            for o in range(DC):
                nc.tensor.matmul(wf_ps, wq_t[:, o, i * P:(i + 1) * P],
                                 m1_t[:, o, :],
                                 start=(o == 0), stop=(o == DC - 1))
            nc.scalar.activation(out=wf_t[:, i, :], in_=wf_ps,
                                 func=AF.Identity)
            # + I : residual is folded into the weights
            nc.vector.tensor_tensor(out=wf_t[:, i, i * P:(i + 1) * P],
                                    in0=wf_t[:, i, i * P:(i + 1) * P],
                                    in1=id_t, op=OP.add)
        bp_ps = ps_f.tile([P, DC], f32, tag="bp", name="bp_ps")
        for d in range(DC):
            for o in range(DC):
                nc.tensor.matmul(bp_ps[:, d:d + 1],
                                 m1_t[:, o, d * P:(d + 1) * P],
                                 bq4b[:, o:o + 1],
                                 start=(o == 0), stop=(o == DC - 1))
        nc.vector.tensor_tensor(out=bfold4, in0=bp_ps, in1=bo4, op=OP.add)
        if dbg:
            for i in range(DC):
                nc.sync.dma_start(out=dWF[i * P:(i + 1) * P, :],
                                  in_=wf_t[:, i, :])

        ps_f.release()
        ps_s.release()
        pa2.release()
        pa.release()

        # ====== Merged pipeline: z -> LN1 -> FFN -> LN2 per l-slice =======
        pc = tc.alloc_tile_pool(name="pc", bufs=1)
        pc2 = tc.alloc_tile_pool(name="pc2", bufs=2)
        ps_st = tc.alloc_tile_pool(name="ps_st", bufs=2, space="PSUM")
        ps_o = tc.alloc_tile_pool(name="ps_o", bufs=2, space="PSUM")
        ps_h = None  # allocated once the O-projection PSUM pool retires

        zq1 = {}

        def emit_O(s):
            # z = x @ (Wfold + I) + bfold ; z^2 on Pool for the LN1 stats
            zq1[s] = []
            for o in range(DC):
                o_ps = ps_o.tile([P, NS], f32, tag="ops", name="o_ps")
                for i in range(DC):
                    nc.tensor.matmul(o_ps, wf_t[:, i, o * P:(o + 1) * P],
                                     xo_list[s][:, i, :],
                                     start=(i == 0), stop=(i == DC - 1))
                zs = z_t[:, o, s * NS:(s + 1) * NS]
                nc.scalar.activation(out=zs, in_=o_ps, func=AF.Identity,
                                     bias=bfold4[:, o:o + 1])
                zq = pc2.tile([P, NS], bf16, tag="zq1", bufs=8, name="zq1")
                nc.gpsimd.tensor_tensor(out=zq, in0=zs, in1=zs, op=OP.mult)
                zq1[s].append(zq)

        def emit_ln1(s):
            # apply writes n1 (un-scaled) in place; gamma/beta live in W1/b1
            def apply(rstd, mhat):
                for d in range(DC):
                    zs = z_t[:, d, s * NS:(s + 1) * NS]
                    td = pc2.tile([P, NS], bf16, tag="td", name="td")
                    nc.vector.tensor_tensor(out=td, in0=zs, in1=rstd,
                                            op=OP.mult)
                    nc.vector.tensor_tensor(out=zs, in0=td, in1=mhat,
                                            op=OP.subtract)
            zq = zq1.pop(s)
            ln_block(pc2, ps_st,
                     lambda d, s=s: z_t[:, d, s * NS:(s + 1) * NS],
                     lambda d: zq[d], apply)

        def emit_h1(s):
            h1_sb = pc.tile([P, PFC, NS], bf16, tag="h1", bufs=2,
                            name="h1_sb")
            for pf in range(PFC):
                h_ps = ps_h.tile([P, NS], f32, tag="hps", name="h_ps")
                for d in range(DC):
                    nc.tensor.matmul(h_ps, w1_t[:, d, pf * P:(pf + 1) * P],
                                     z_t[:, d, s * NS:(s + 1) * NS],
                                     start=(d == 0), stop=(d == DC - 1))
                nc.scalar.activation(out=h1_sb[:, pf, :], in_=h_ps,
                                     func=AF.Relu, bias=b116[:, pf:pf + 1])
            return h1_sb

        def emit_f(s, h1_sb, c0=0, w=NS):
            z2 = pc2.tile([P, DC, NS], bf16, tag="z2", name="z2")
            zq2 = pc2.tile([P, DC, NS], bf16, tag="zq2", name="zq2")
            for o in range(DC):
                f_ps = ps_h.tile([P, NS], f32, tag="fps", name="f_ps")[:, 0:w]
                # y1 = g*n1 + b re-enters via diag(g); bias b+b2 on the evac
                nc.tensor.matmul(f_ps, gd_t[:, o, :],
                                 z_t[:, o, s * NS + c0:s * NS + c0 + w],
                                 start=True, stop=False)
                for k in range(PFC):
                    nc.tensor.matmul(f_ps, w2_t[:, k, o * P:(o + 1) * P],
                                     h1_sb[:, k, c0:c0 + w],
                                     start=False, stop=(k == PFC - 1))
                nc.scalar.activation(out=z2[:, o, c0:c0 + w], in_=f_ps,
                                     func=AF.Identity, bias=b24[:, o:o + 1])
                nc.scalar.activation(out=zq2[:, o, c0:c0 + w], in_=f_ps,
                                     func=AF.Square, bias=b24[:, o:o + 1])
            return z2, zq2

        def emit_ln2t(z2, zq2, s, c0=0, w=NS, pool_split=False):
            yo = pc2.tile([P, DC, NS], f32, tag="yo", name="yo")

            def apply(rstd, mhat):
                for d in range(DC):
                    # on the final tail blocks, odd chunks run on the idle
                    # Pool engine so the apply isn't DVE-serialized
                    eng = nc.gpsimd if (pool_split and d % 2) else nc.vector
                    td = pc2.tile([P, NS], bf16, tag="td",
                                  name="td")[:, 0:w]
                    eng.tensor_tensor(out=td, in0=z2[:, d, c0:c0 + w],
                                      in1=rstd, op=OP.mult)
                    eng.tensor_tensor(out=td, in0=td, in1=mhat,
                                      op=OP.subtract)
                    nc.scalar.activation(out=yo[:, d, c0:c0 + w], in_=td,
                                         func=AF.Identity,
                                         scale=g4[:, d:d + 1],
                                         bias=lnb4[:, d:d + 1])
                    nc.scalar.dma_start(
                        out=yT[d * P:(d + 1) * P,
                               s * NS + c0:s * NS + c0 + w],
                        in_=yo[:, d, c0:c0 + w])
            ln_block(pc2, ps_st,
                     lambda d, z2=z2: z2[:, d, c0:c0 + w],
                     lambda d, zq2=zq2: zq2[:, d, c0:c0 + w], apply,
                     c0=c0, w=w)

        # software-pipelined emission: LN work rides between GEMM groups
        emit_O(0)
        emit_O(1)
        emit_ln1(0)
        emit_O(2)
        emit_ln1(1)
        emit_O(3)
        ps_o.release()
        ps_h = tc.alloc_tile_pool(name="ps_h", bufs=2, space="PSUM")
        h0 = emit_h1(0)
        emit_ln1(2)
        t0 = emit_f(0, h0)
        h1_ = emit_h1(1)
        emit_ln1(3)
        t1 = emit_f(1, h1_)
        h2 = emit_h1(2)
        emit_ln2t(*t0, 0)
        t2 = emit_f(2, h2)
        h3 = emit_h1(3)
        emit_ln2t(*t1, 1)
        # T2 before f(3): its stats follow h1(3) on the PE and its DVE chain
        # hides under f(3)'s matmuls.
        emit_ln2t(*t2, 2)
        # f(3) in halves so T3a's chain hides under f(3)'s second half
        t3a = emit_f(3, h3, c0=0, w=NS // 2)
        emit_ln2t(*t3a, 3, c0=0, w=NS // 2, pool_split=True)
        t3b = emit_f(3, h3, c0=NS // 2, w=NS // 2)
        emit_ln2t(*t3b, 3, c0=NS // 2, w=NS // 2, pool_split=True)
        if dbg:
            for c in range(DC):
                nc.sync.dma_start(out=dZ[c * P:(c + 1) * P, :],
                                  in_=z_t[:, c, :])

        ps_h.release()
        ps_st.release()
        pc2.release()
        pc.release()
        pab.release()
        gp.release()

    nc.compile()
    return nc


def get_nc(dbg=False, repeat=1):
    key = f"nc_{dbg}_{repeat}"
    if key not in _CACHE:
        _CACHE[key] = _build_nc(dbg=dbg, repeat=repeat)
    return _CACHE[key]


def _host_prep(inputs):
    BF = ml_dtypes.bfloat16
    f = lambda a: np.ascontiguousarray(np.asarray(a), dtype=np.float32)
    fb = lambda a: np.ascontiguousarray(
        np.asarray(a, dtype=np.float32).astype(BF))
    W1 = np.asarray(inputs["W1"], dtype=np.float32)
    g = np.asarray(inputs["ln_g"], dtype=np.float32)
    b = np.asarray(inputs["ln_b"], dtype=np.float32)
    gd = np.zeros((D, P), np.float32)
    for c in range(DC):
        gd[c * P:(c + 1) * P, :] = np.diag(g[c * P:(c + 1) * P])
    shared = {
        "wkT": fb(np.asarray(inputs["Wk"]).T),
        "wvT": fb(np.asarray(inputs["Wv"]).T),
        "woT": fb(np.asarray(inputs["Wo"]).T),
        "wqN": fb(np.asarray(inputs["Wq"])),
        "w1T": fb((W1 * g[None, :]).T),
        "w2T": fb(np.asarray(inputs["W2"]).T),
        "idm": fb(np.eye(P, dtype=np.float32)),
        "gdg": fb(gd),
        "bqc": fb(inputs["bq"]),
        "bk": f(inputs["bk"]), "bv": f(inputs["bv"]),
        "bo": f(inputs["bo"]),
        "b1": f(W1 @ b + np.asarray(inputs["b1"], np.float32)),
        "b2": f(np.asarray(inputs["b2"], np.float32) + b),
        "lng": f(g), "lnb": f(b),
    }
    question = np.asarray(inputs["question"], dtype=np.float32)
    query = np.asarray(inputs["query"], dtype=np.float32)
    srcs = (question, query)
    in_maps = []
    for idx in range(2):
        for bi in range(B):
            in_maps.append({
                "x_own": fb(srcs[idx][bi].T),
                "x_oth": fb(srcs[1 - idx][bi].T),
                **shared,
            })
    return in_maps


def run_sharded(inputs, trace=False, dbg=False, repeat=1):
    nc = get_nc(dbg=dbg, repeat=repeat)
    in_maps = _host_prep(inputs)
    res = bass_utils.run_bass_kernel_spmd(nc, in_maps,
                                          core_ids=list(range(8)),
                                          trace=trace)
    out = np.empty((B, L, 2 * D), np.float32)
    for c in range(8):
        idx, b = divmod(c, B)
        out[b, :, idx * D:(idx + 1) * D] = res.results[c]["yT"].T
    return out, res


def kernel(**inputs):
    out, _ = run_sharded(inputs)
    return out


if __name__ == "__main__":
    # smoke build
    get_nc()
    print("build ok")
